# revision 62
# baseline (speedup 1.0000x reference)
"""GPT block (LN -> causal MHA -> LN -> MLP) on 8 TRN2 NeuronCores.

Sharding: each core owns one (batch, query-chunk-pair). B=4 batches x 2
chunk-pairs = 8 cores. Chunk pairs are zig-zag ({0,3} / {1,2}) over four
512-row chunks of T=2048 so attention work balances. Each core recomputes
K/V for the full sequence locally (no collectives), runs flash-style
attention for its 1024 query rows, then the MLP for the same rows.

All activations live feature-on-partition ("transposed"); the host
pre-transposes x (bf16) and swizzles every weight into its exact SBUF
panel layout (per-partition-contiguous DMA). Causality: block
permutation + exp scale/bias inputs for whole-block allow/deny + static
0/1 masks for the diagonal blocks (applied post-exp on Vector).

v7 schedule (QKV runs fp8 DoubleRow: ln1_t is fp8, q/k/v weights are
host-prescaled by 256/64 into fp8 range and descaled at the psum
store; 256-deep reduction halves QKV instructions):
  x preload + LN1 (chunk-pipelined, bf16 norm on Vector, squares and
  stats col-tiled on PE) -> QKV subset (q, k chunks {0,2}, v tiles for
  attention-A) ->
  [attention A || remaining K/V GEMMs] ->
  [attention B || attnproj-A + LN2-A + FC-A (gelu deferred)] ->
  [attnproj-B + LN2-B + FC-B || proj-A] -> proj-B.
Attention processes HEAD PAIRS: the two 64-contraction S matmuls of a
pair run concurrently in disjoint PE row groups (tile_position auto),
scores ping-pong between two 2-bank psum tiles so exp on Scalar stays
saturated, es is fp8e4, and AV uses fp8 DoubleRow matmuls (256-deep
reduction, half the instructions). Softmax denominators ride the 65th
v-row; the reciprocal is staged through SBUF (DVE reciprocal from PSUM
reads the wrong partition). The Scalar engine runs ONLY exp inside
attention windows (ACT table reloads cost 1.3us): QKV remnant stores,
FC bias adds and LN2 stat scaling run on Vector/GpSimd, and FC-A's
gelu runs later, gated behind attention-B via a zero bias tile.
"""

import numpy as np
import ml_dtypes

BF = ml_dtypes.bfloat16

E = 1024          # embedding
T = 2048          # sequence
B = 4             # batch
NH = 16           # heads
D = 64            # head dim
HID = 4096        # mlp hidden
KT = E // 128     # k-tiles over embedding (8)
QSC = 256.0       # fp8 scale folded into q weights (descaled at store)
KSC = 64.0        # fp8 scale folded into k/v weights
CH = 512          # chunk rows
NEG = -1.0e9
EPS = 1e-5

_CACHE = {}


def _build_program():
    import concourse.bass as bass
    import concourse.tile as tile
    from concourse import bacc, mybir

    f32 = mybir.dt.float32
    f32r = mybir.dt.float32r
    bf16 = mybir.dt.bfloat16
    f8 = mybir.dt.float8e4
    DR = mybir.MatmulPerfMode.DoubleRow
    AF = mybir.ActivationFunctionType
    ALU = mybir.AluOpType

    nc = bacc.Bacc()

    # All weights/x arrive host-swizzled into per-partition-contiguous
    # panel layouts so every panel DMA is 1 descriptor per partition.
    x_d = nc.declare_dram_parameter("x_swz", [128, 4, KT, CH], bf16, isOutput=False)
    w_qk_d = nc.declare_dram_parameter("w_qk", [128, 8, KT, 256], f8, isOutput=False)
    w_v_d = nc.declare_dram_parameter("w_v", [128, 2, KT, CH], f8, isOutput=False)
    b_qk_d = nc.declare_dram_parameter("b_qk", [2 * E, 1], f32, isOutput=False)
    b_v_d = nc.declare_dram_parameter("b_v_bc", [128, 2, 8, D], bf16, isOutput=False)
    w_ap_d = nc.declare_dram_parameter("w_ap", [128, 4, KT, 256], bf16, isOutput=False)
    b_ap_d = nc.declare_dram_parameter("b_ap", [E, 1], f32, isOutput=False)
    w_fc_d = nc.declare_dram_parameter("w_fc", [128, 8, KT, CH], bf16, isOutput=False)
    b_fc_d = nc.declare_dram_parameter("b_fc", [HID, 1], f32, isOutput=False)
    w_pr_d = nc.declare_dram_parameter("w_proj", [128, 8, 4, E], bf16, isOutput=False)
    b_pr_d = nc.declare_dram_parameter("b_proj", [E, 1], f32, isOutput=False)
    pm_d = nc.declare_dram_parameter("pmask", [2, 128, 2 * CH], bf16, isOutput=False)
    sA_s_d = nc.declare_dram_parameter("sA_scale", [128, 1], f32, isOutput=False)
    sA_b_d = nc.declare_dram_parameter("sA_bias", [128, 1], f32, isOutput=False)
    sB_s_d = nc.declare_dram_parameter("sB_scale", [128, 3], f32, isOutput=False)
    sB_b_d = nc.declare_dram_parameter("sB_bias", [128, 3], f32, isOutput=False)
    out_d = nc.declare_dram_parameter("outT", [E, 2 * CH], f32, isOutput=True)

    with tile.TileContext(nc) as tc:
        from contextlib import ExitStack

        stack = ExitStack()
        with stack:
            # ---- long-lived left-side pools (LIFO nesting) ----
            const = stack.enter_context(tc.tile_pool(name="const", bufs=1))
            aBp = stack.enter_context(tc.tile_pool(name="aBp", bufs=1))
            h2p = stack.enter_context(tc.tile_pool(name="h2p", bufs=1))
            # ---- right-side ----
            x2p = stack.enter_context(tc.tile_pool(name="x2p", bufs=1, side="right"))

            ones_col_bf = const.tile([128, 1], bf16)
            nc.vector.memset(ones_col_bf[:], 1.0 / E)
            ones_col_f = const.tile([128, 1], f32)
            nc.vector.memset(ones_col_f[:], 1.0)
            eps_t = const.tile([1, 1], f32)
            nc.vector.memset(eps_t[:], EPS)
            gate_t = const.tile([128, 1], f32)

            pmask = const.tile([128, 2, 2 * CH], bf16)
            nc.sync.dma_start(pmask[:], pm_d.rearrange("v p n -> p v n"))
            pm_i = pmask.rearrange("p v (i c) -> p v i c", i=2)
            sA_s = const.tile([128, 1], f32)
            nc.sync.dma_start(sA_s[:], sA_s_d[:])
            sA_b = const.tile([128, 1], f32)
            nc.sync.dma_start(sA_b[:], sA_b_d[:])
            sB_s = const.tile([128, 3], f32)
            nc.sync.dma_start(sB_s[:], sB_s_d[:])
            sB_b = const.tile([128, 3], f32)
            nc.sync.dma_start(sB_b[:], sB_b_d[:])

            b_qk = const.tile([128, 16, 1], f32)
            nc.sync.dma_start(b_qk[:], b_qk_d.rearrange("(k p) o -> p k o", p=128))
            b_v = const.tile([128, 2, 8, D], bf16)
            nc.sync.dma_start(b_v[:], b_v_d[:])
            b_ap = const.tile([128, KT, 1], f32)
            nc.sync.dma_start(b_ap[:], b_ap_d.rearrange("(k p) o -> p k o", p=128))
            b_fc = const.tile([128, 32, 1], f32)
            nc.sync.dma_start(b_fc[:], b_fc_d.rearrange("(k p) o -> p k o", p=128))
            b_pr = const.tile([128, KT, 1], f32)
            nc.sync.dma_start(b_pr[:], b_pr_d.rearrange("(k p) o -> p k o", p=128))

            attnTB = aBp.tile([128, KT, CH], bf16)
            h2T = h2p.tile([128, KT, 2 * CH], bf16)
            x2A = x2p.tile([128, KT, CH], bf16)
            x2s = [x2A, None]

            # window-scoped pools (opened/closed in LIFO order)
            aAp = stack.enter_context(tc.tile_pool(name="aAp", bufs=1))
            attnTA = aAp.tile([128, KT, CH], bf16)
            attnTs = [attnTA, attnTB]

            atsb_ctx = ExitStack()
            atsb = atsb_ctx.enter_context(tc.tile_pool(name="atsb", bufs=4))

            # ---------------- Phase 1: LN1 (chunk-pipelined) ----------------
            # QKV panel pool + psum opened BEFORE the LN1 scratch pools so
            # the panel DMAs have no SBUF anti-dependency on LN1 work.
            wqk_ctx = ExitStack()
            wqk_pool = wqk_ctx.enter_context(tc.tile_pool(name="wqk", bufs=3))
            qkps_ctx = ExitStack()
            qkps = qkps_ctx.enter_context(
                tc.tile_pool(name="qkps", bufs=4, space="PSUM"))

            ln1_ctx = ExitStack()
            ln1 = ln1_ctx.enter_context(tc.tile_pool(name="ln1", bufs=1))
            ln1_t = ln1.tile([128, KT, T], f8)

            # x preload pool sits above ln1 so it can close after LN1 and
            # free its 32KB before the attention-side pools grow.
            xcp_ctx = ExitStack()
            xcp = xcp_ctx.enter_context(tc.tile_pool(name="xcp", bufs=1))
            xc = xcp.tile([128, 4, KT, CH], bf16)
            for ch in range(4):
                nc.sync.dma_start(xc[:, ch], x_d[:, ch])

            with tc.tile_pool(name="ln1ps", bufs=1, space="PSUM") as lnps, \
                 tc.tile_pool(name="ln1sb", bufs=1) as lnsb:
                state = {}

                def ln1_stats(ch):
                    xt = xc[:, ch]
                    st = lnps.tile([33, CH], f32, tag="stat", bufs=2,
                                   name=f"l1st{ch}")
                    mu_ps = st[0:1, :]
                    ss_ps = st[32:33, :]
                    sq = lnsb.tile([128, KT, CH], bf16, tag="sq", bufs=2,
                                   name=f"l1sq{ch}")
                    nc.scalar.square(sq[:, 0:4, :], xt[:, 0:4, :])
                    nc.scalar.square(sq[:, 4:8, :], xt[:, 4:8, :])
                    for kt in range(KT):
                        nc.tensor.matmul(mu_ps, ones_col_bf[:], xt[:, kt, :],
                                         start=(kt == 0), stop=(kt == KT - 1),
                                         tile_position=(0, 0))
                        nc.tensor.matmul(ss_ps, ones_col_bf[:], sq[:, kt, :],
                                         start=(kt == 0), stop=(kt == KT - 1),
                                         tile_position=(0, 32))
                    state[ch] = (mu_ps, ss_ps)

                def ln1_norm(ch):
                    mu_ps, ss_ps = state.pop(ch)
                    xt = xc[:, ch]
                    mu = lnsb.tile([1, CH], f32, tag="row", bufs=5)
                    nc.vector.tensor_copy(mu[:], mu_ps[:])
                    msq = lnsb.tile([1, CH], f32, tag="row", bufs=5)
                    nc.vector.tensor_mul(msq[:], mu[:], mu[:])
                    var = lnsb.tile([1, CH], f32, tag="row", bufs=5)
                    nc.vector.tensor_sub(var[:], ss_ps[:], msq[:])
                    sd = lnsb.tile([1, CH], f32, tag="row", bufs=5)
                    nc.scalar.activation(sd[:], var[:], AF.Sqrt,
                                         bias=eps_t[:])
                    rstd = lnsb.tile([1, CH], f32, tag="row", bufs=5)
                    nc.vector.reciprocal_approx_fast(rstd[:], sd[:])
                    rstd_h = lnsb.tile([1, CH], bf16, tag="rowh", bufs=4)
                    nc.vector.tensor_copy(rstd_h[:], rstd[:])
                    nmr = lnsb.tile([1, CH], bf16, tag="rowh", bufs=4)
                    nc.vector.scalar_tensor_tensor(
                        nmr[:], mu[:], -1.0, rstd[:], ALU.mult, ALU.mult)
                    a_bc = lnsb.tile([128, CH], bf16, tag="bc", bufs=3)
                    nc.gpsimd.partition_broadcast(a_bc[:], rstd_h[:])
                    c_bc = lnsb.tile([128, CH], bf16, tag="bc", bufs=3)
                    nc.gpsimd.partition_broadcast(c_bc[:], nmr[:])
                    for kt in range(KT):
                        t1 = lnsb.tile([128, CH], bf16, tag="t1", bufs=4)
                        nc.vector.tensor_mul(t1[:], xt[:, kt, :], a_bc[:])
                        nc.vector.tensor_add(
                            ln1_t[:, kt, ch * CH:(ch + 1) * CH],
                            t1[:], c_bc[:])

                for ch in range(5):
                    if ch < 4:
                        ln1_stats(ch)
                    if ch >= 1:
                        ln1_norm(ch - 1)
            xcp_ctx.close()

            # ---------------- Phase 2: QKV (A-priority subset) --------------
            # Main phase computes q (both chunks), k for kv chunks {0,2} and
            # v for kv tiles {0-3, 8-11} -- everything attention-A needs.
            # The rest (k chunks {1,3}, v tiles {4-7, 12-15}) is emitted by
            # qkv_remnant() as PE filler inside the attention-A window.
            qA_ctx = ExitStack()
            qAp = qA_ctx.enter_context(tc.tile_pool(name="qAp", bufs=1))
            qTA = qAp.tile([128, KT, CH], bf16)
            gpA = stack.enter_context(
                tc.tile_pool(name="gpA", bufs=1, side="right"))
            gTA = gpA.tile([128, 32, CH], bf16)
            qkv_ctx = ExitStack()
            qkvp = qkv_ctx.enter_context(
                tc.tile_pool(name="qkvp", bufs=1, side="right"))
            qTB = qkvp.tile([128, KT, CH], bf16)
            qTs = [qTA, qTB]
            kT = qkvp.tile([128, KT, T], bf16)
            v_aug = qkvp.tile([128, 8, 2, NH * 65], f8)
            v5 = v_aug.rearrange("p j i (h w) -> p j i h w", h=NH)  # w=68, cols 65-67 unused

            def qkv_k_group(panel, g, mm, nq, pool=None):
                mt = 2 * g + mm
                is_q = mt < 8
                dt_idx = mt if is_q else mt - 8
                dsc = 1.0 / (QSC if is_q else KSC)
                ps = (pool or qkps).tile([128, CH], f32, tag="ps",
                                         bufs=4 if pool is None else 2)
                for kp in range(KT // 2):
                    nc.tensor.matmul(
                        ps[:],
                        panel[:, 2 * kp:2 * kp + 2, mm * 128:(mm + 1) * 128],
                        ln1_t[:, 2 * kp:2 * kp + 2, nq * CH:(nq + 1) * CH],
                        start=(kp == 0), stop=(kp == KT // 2 - 1),
                        perf_mode=DR)
                dst_ap = (qTs[nq][:, dt_idx, :] if is_q else
                          kT[:, dt_idx, nq * CH:(nq + 1) * CH])
                if pool is None:
                    nc.scalar.activation(dst_ap, ps[:], AF.Identity,
                                         bias=b_qk[:, mt, 0:1], scale=dsc)
                else:
                    # remnant runs inside the exp window: keep Scalar
                    # exp-only (ACT table reloads cost 1.3us each)
                    nc.vector.tensor_scalar(dst_ap, ps[:], dsc,
                                            b_qk[:, mt, 0:1],
                                            ALU.mult, ALU.add)

            def qkv_v_group(g, mv, pool=None):
                ps = (pool or qkps).tile([128, CH], f32, tag="ps",
                                         bufs=4 if pool is None else 2)
                for kp in range(KT // 2):
                    nc.tensor.matmul(
                        ps[:],
                        ln1_t[:, 2 * kp:2 * kp + 2, mv * 128:(mv + 1) * 128],
                        v_panels[g][:, 2 * kp:2 * kp + 2, :],
                        start=(kp == 0), stop=(kp == KT // 2 - 1),
                        perf_mode=DR)
                ps3 = ps.rearrange("p (h w) -> p h w", h=8)
                nc.vector.scalar_tensor_tensor(
                    v5[:, mv // 2, mv % 2, g * 8:(g + 1) * 8, 0:64],
                    ps3[:], 1.0 / KSC, b_v[:, g], ALU.mult, ALU.add)
                nc.vector.memset(
                    v5[:, mv // 2, mv % 2, g * 8:(g + 1) * 8, 64:65], 1.0)

            v_panels = {}
            for g in range(4):          # q
                panel = wqk_pool.tile([128, KT, 256], f8, tag="w")
                nc.sync.dma_start(panel[:], w_qk_d[:, g])
                for mm in range(2):
                    for nq in range(2):
                        qkv_k_group(panel, g, mm, nq)
            for g in range(4, 8):       # k, chunks 0/2 (attention-A set)
                panel = wqk_pool.tile([128, KT, 256], f8, tag="w")
                nc.sync.dma_start(panel[:], w_qk_d[:, g])
                for mm in range(2):
                    for nq in (0, 2):
                        qkv_k_group(panel, g, mm, nq)
            for g in range(2):          # v, tiles 0-3 / 8-11
                panel = wqk_pool.tile([128, KT, CH], f8, tag="wv", bufs=2,
                                      name=f"wv{g}")
                nc.sync.dma_start(panel[:], w_v_d[:, g])
                v_panels[g] = panel
                for mv in (0, 1, 2, 3, 8, 9, 10, 11):
                    qkv_v_group(g, mv)

            def qkv_remnant():
                """K chunks {1,3} + V tiles {4-7, 12-15}: PE filler during
                attention-A. v_panels stay resident from the main phase."""
                for g in range(4, 8):
                    panel = wqk_pool.tile([128, KT, 256], f8, tag="w",
                                          name=f"rk{g}")
                    nc.sync.dma_start(panel[:], w_qk_d[:, g])
                    for mm in range(2):
                        for nq in (1, 3):
                            qkv_k_group(panel, g, mm, nq, pool=gemmps)
                            yield
                for g in range(2):
                    panel = wqk_pool.tile([128, KT, CH], f8, tag="wv",
                                          bufs=2, name=f"rv{g}")
                    nc.sync.dma_start(panel[:], w_v_d[:, g])
                    v_panels[g] = panel
                    for mv in (4, 5, 6, 7, 12, 13, 14, 15):
                        qkv_v_group(g, mv, pool=gemmps)
                        yield

            qkps_ctx.close()

            # gemm-filler psum (2 banks) + attention psum (s 4 + av 2)
            gemmps_ctx = ExitStack()
            gemmps = gemmps_ctx.enter_context(
                tc.tile_pool(name="gemmps", bufs=1, space="PSUM"))
            atps_ctx = ExitStack()
            atps = atps_ctx.enter_context(
                tc.tile_pool(name="atps", bufs=1, space="PSUM"))

            # ============ attention machinery ============
            pairs_a = [(0, 1, ("diag", 0)), (2, 3, ("diag", 1)),
                       (8, 9, ("drv", "A", 0)), (10, 11, ("drv", "A", 0))]
            pairs_b = [(4, 5, ("diag", 0)), (6, 7, ("diag", 1)),
                       (0, 1, ("drv", "B", 0)), (2, 3, ("drv", "B", 0)),
                       (8, 9, ("drv", "B", 1)), (10, 11, ("drv", "B", 1)),
                       (12, 13, ("drv", "B", 2)), (14, 15, ("drv", "B", 2))]

            avs = {}
            ess = {}

            def emit_s_exp(slot, hp, j, seq):
                """S for head pair (2hp, 2hp+1), kv pair j. The four S
                matmuls alternate row-groups {0,1}/{2,3} and target four
                different psum banks, so adjacent ones run concurrently."""
                t0, t1, mk = seq[j]
                es = atsb.tile([128, 2, 2 * CH], f8, tag="es", bufs=3)
                for i, t in enumerate((t0, t1)):
                    # per-tile score half: 2 psum banks, ring of 2 so the
                    # next S pair overlaps the current exp on Scalar
                    s_i = atps.tile([128, 2 * CH], f32, tag="s", bufs=2,
                                    name="sU")
                    nc.tensor.matmul(
                        s_i[:, 0:CH],
                        kT[0:64, hp, t * 128:(t + 1) * 128],
                        qTs[slot][0:64, hp, :], start=True, stop=True)
                    nc.tensor.matmul(
                        s_i[:, CH:2 * CH],
                        kT[64:128, hp, t * 128:(t + 1) * 128],
                        qTs[slot][64:128, hp, :], start=True, stop=True)
                    if mk[0] == "diag":
                        nc.scalar.activation(es[:, i, :], s_i[:], AF.Exp)
                    else:
                        sc = sA_s if mk[1] == "A" else sB_s
                        bi = sA_b if mk[1] == "A" else sB_b
                        idx = mk[2]
                        nc.scalar.activation(
                            es[:, i, :], s_i[:], AF.Exp,
                            bias=bi[:, idx:idx + 1],
                            scale=sc[:, idx:idx + 1])
                if mk[0] == "diag":
                    for par in range(2):
                        nc.vector.tensor_mul(
                            es[:, :, par * CH:(par + 1) * CH],
                            es[:, :, par * CH:(par + 1) * CH],
                            pm_i[:, mk[1]])
                ess[(hp, slot, j)] = es

            def emit_pv(slot, hp, j, seq):
                """fp8 DoubleRow AV: one matmul per head covers both kv
                tiles of the pair (256-deep reduction)."""
                t0, t1, mk = seq[j]
                jp = t0 // 2
                npairs = len(seq)
                if j == 0:
                    avs[(hp, slot, 0)] = atps.tile(
                        [65, CH], f32, tag="av", bufs=2,
                        name=f"avE{hp}_{slot}")
                    avs[(hp, slot, 1)] = atps.tile(
                        [65, CH], f32, tag="av", bufs=2,
                        name=f"avO{hp}_{slot}")
                es = ess.pop((hp, slot, j))
                for par in range(2):
                    h = 2 * hp + par
                    nc.tensor.matmul(
                        avs[(hp, slot, par)][:],
                        v_aug[:, jp, :, h * 65:(h + 1) * 65],
                        es[:, :, par * CH:(par + 1) * CH],
                        start=(j == 0), stop=(j == npairs - 1),
                        perf_mode=DR)
                if j == npairs - 1:
                    for par in range(2):
                        h = 2 * hp + par
                        ro = (h % 2) * 64
                        out_ps = avs.pop((hp, slot, par))
                        den = atsb.tile([1, CH], f32, tag="dn", bufs=2)
                        nc.vector.tensor_copy(den[:], out_ps[64:65, :])
                        rec = atsb.tile([1, CH], f32, tag="rc", bufs=2)
                        nc.vector.reciprocal_approx_fast(rec[:], den[:])
                        bc64 = atsb.tile([64, CH], f32, tag="bcr", bufs=2)
                        nc.gpsimd.partition_broadcast(bc64[:], rec[:])
                        nc.vector.tensor_mul(
                            attnTs[slot][ro:ro + 64, hp, :],
                            out_ps[0:64, :], bc64[:])

            def attn_stream(slot):
                """Yields once per unit, BEFORE the unit's S matmuls, so
                interleaved filler work sits ahead of the next stalling S
                in the PE FIFO (covers the exp latency)."""
                seq = pairs_a if slot == 0 else pairs_b
                units = [(hp, j) for hp in range(NH // 2)
                         for j in range(len(seq))]
                prev = None
                for hp, j in units:
                    yield
                    if prev is not None:
                        emit_pv(slot, prev[0], prev[1], seq)
                    emit_s_exp(slot, hp, j, seq)
                    prev = (hp, j)
                emit_pv(slot, prev[0], prev[1], seq)

            def ap_ln2_stream(slot, mlsb, mlps):
                """attnproj + residual -> LN2 for one slot."""
                nq = slot
                for mp in range(4):
                    wpan = mlsb.tile([128, KT, 256], bf16, tag="wap", bufs=2,
                                     name=f"wap{slot}_{mp}")
                    nc.sync.dma_start(wpan[:], w_ap_d[:, mp])
                    for half in range(2):
                        m = 2 * mp + half
                        ps = mlps.tile([128, CH], f32, tag="ps", bufs=2,
                                       name=f"ap{slot}_{m}")
                        for kt in range(KT):
                            nc.tensor.matmul(
                                ps[:], wpan[:, kt, half * 128:(half + 1) * 128],
                                attnTs[slot][:, kt, :],
                                start=(kt == 0), stop=(kt == KT - 1))
                        xq = mlsb.tile([128, CH], bf16, tag="xq", bufs=2,
                                       name=f"xq{slot}_{m}")
                        nc.sync.dma_start(xq[:], x_d[:, nq, m, :])
                        nc.vector.scalar_tensor_tensor(
                            x2s[nq][:, m, :], ps[:],
                            b_ap[:, m, 0:1], xq[:], ALU.add, ALU.add)
                        yield
                # LN2 (stats borrow partitions 0/32 of a [128, CH] psum tile;
                # x2 is bf16 so the stats matmuls run in bf16 directly)
                stat_t = mlps.tile([128, CH], f32, tag="ps", bufs=2,
                                   name=f"l2s{slot}")
                mu_ps = stat_t[0:1, :]
                ss_ps = stat_t[32:33, :]
                src = x2s[nq][:, :, :]
                for kt in range(KT):
                    sq = mlsb.tile([128, CH], bf16, tag="sq2", bufs=2,
                                   name=f"l2sq{slot}_{kt}")
                    nc.gpsimd.tensor_mul(sq[:], src[:, kt, :], src[:, kt, :])
                    nc.tensor.matmul(mu_ps, ones_col_bf[:], src[:, kt, :],
                                     start=(kt == 0), stop=(kt == KT - 1),
                                     tile_position=(0, 0))
                    nc.tensor.matmul(ss_ps, ones_col_bf[:], sq[:],
                                     start=(kt == 0), stop=(kt == KT - 1),
                                     tile_position=(0, 32))
                    if kt % 2 == 1:
                        yield
                mu = mlsb.tile([1, CH], f32, tag="row2", bufs=7)
                nc.vector.tensor_copy(mu[:], mu_ps)
                ms = mlsb.tile([1, CH], f32, tag="row2", bufs=7)
                nc.vector.tensor_copy(ms[:], ss_ps)
                msq = mlsb.tile([1, CH], f32, tag="row2", bufs=7)
                nc.vector.tensor_mul(msq[:], mu[:], mu[:])
                var = mlsb.tile([1, CH], f32, tag="row2", bufs=7)
                nc.vector.tensor_sub(var[:], ms[:], msq[:])
                sd = mlsb.tile([1, CH], f32, tag="row2", bufs=7)
                nc.scalar.activation(sd[:], var[:], AF.Sqrt, bias=eps_t[:])
                rstd = mlsb.tile([1, CH], f32, tag="row2", bufs=7)
                nc.vector.reciprocal_approx_fast(rstd[:], sd[:])
                rstd_h = mlsb.tile([1, CH], bf16, tag="row2h", bufs=4)
                nc.vector.tensor_copy(rstd_h[:], rstd[:])
                nmr = mlsb.tile([1, CH], bf16, tag="row2h", bufs=4)
                nc.vector.scalar_tensor_tensor(
                    nmr[:], mu[:], -1.0, rstd[:], ALU.mult, ALU.mult)
                a_bc = mlsb.tile([128, CH], bf16, tag="bc2", bufs=2)
                nc.gpsimd.partition_broadcast(a_bc[:], rstd_h[:])
                c_bc = mlsb.tile([128, CH], bf16, tag="bc2", bufs=2)
                nc.gpsimd.partition_broadcast(c_bc[:], nmr[:])
                for kt in range(KT):
                    t1 = mlsb.tile([128, CH], bf16, tag="t12", bufs=2)
                    nc.vector.tensor_mul(t1[:], src[:, kt, :], a_bc[:])
                    nc.vector.tensor_add(
                        h2T[:, kt, nq * CH:(nq + 1) * CH], t1[:], c_bc[:])
                    if kt % 2 == 1:
                        yield

            def fc_stream(slot, g_t, mlsb, mlps, defer_gelu=False):
                nq = slot
                for mg in range(8):
                    panel = mlsb.tile([128, KT, CH], bf16, tag="wfc", bufs=2,
                                      name=f"fc{slot}_{mg}")
                    nc.sync.dma_start(panel[:], w_fc_d[:, mg])
                    for mm in range(4):
                        ps = mlps.tile([128, CH], f32, tag="ps", bufs=2,
                                       name=f"fc{slot}_{mg}_{mm}")
                        for kt in range(KT):
                            nc.tensor.matmul(
                                ps[:], panel[:, kt, mm * 128:(mm + 1) * 128],
                                h2T[:, kt, nq * CH:(nq + 1) * CH],
                                start=(kt == 0), stop=(kt == KT - 1))
                        mt = mg * 4 + mm
                        if defer_gelu:
                            nc.vector.tensor_scalar_add(
                                g_t[:, mt, :], ps[:], b_fc[:, mt, 0:1])
                        else:
                            nc.scalar.activation(
                                g_t[:, mt, :], ps[:],
                                AF.Gelu, bias=b_fc[:, mt, 0:1])
                        yield

            def proj_stream(slot, g_t, prps, prsb):
                for half in range(2):
                    ms = list(range(half * 4, (half + 1) * 4))
                    pss = {m: prps.tile([128, CH], f32, tag="ps", bufs=4,
                                        name=f"pr{slot}_{half}_{m}")
                           for m in ms}
                    for kg in range(8):
                        panel = prsb.tile([128, 4, E], bf16, tag="w", bufs=3,
                                          name=f"prw{slot}_{half}_{kg}")
                        nc.sync.dma_start(panel[:], w_pr_d[:, kg])
                        for kk in range(4):
                            kt = kg * 4 + kk
                            for m in ms:
                                nc.tensor.matmul(
                                    pss[m][:],
                                    panel[:, kk, m * 128:(m + 1) * 128],
                                    g_t[:, kt, :],
                                    start=(kt == 0), stop=(kt == 31),
                                    skip_group_check=True)
                        yield
                    for m in ms:
                        ot = prsb.tile([128, CH], f32, tag="ot", bufs=4,
                                       name=f"ot{slot}_{half}_{m}")
                        nc.vector.scalar_tensor_tensor(
                            ot[:], pss[m][:], b_pr[:, m, 0:1],
                            x2s[slot][:, m, :], ALU.add, ALU.add)
                        nc.sync.dma_start(
                            out_d[m * 128:(m + 1) * 128,
                                  slot * CH:(slot + 1) * CH],
                            ot[:])
                        yield

            def chain(*gens):
                for g in gens:
                    for _ in g:
                        yield

            def interleave(main, filler, ratio):
                budget = 0.0
                for _ in main:
                    budget += ratio
                    while budget >= 1.0 and filler is not None:
                        budget -= 1.0
                        try:
                            next(filler)
                        except StopIteration:
                            filler = None
                if filler is not None:
                    for _ in filler:
                        pass

            # ---- window A: attention A || qkv remnant ----
            interleave(attn_stream(0), qkv_remnant(), 33 / 32.0)
            qA_ctx.close()
            ln1_ctx.close()
            wqk_ctx.close()

            # ---- window B: attention B || ap-A + LN2-A + FC-A ----
            w2_ctx = ExitStack()
            w2sb = w2_ctx.enter_context(tc.tile_pool(name="w2sb", bufs=1))
            interleave(attn_stream(1),
                       chain(ap_ln2_stream(0, w2sb, gemmps),
                             fc_stream(0, gTA, w2sb, gemmps,
                                       defer_gelu=True)),
                       49 / 64.0)
            w2_ctx.close()
            atsb_ctx.close()
            atps_ctx.close()
            qkv_ctx.close()

            # ---- window 3: ap-B + LN2-B, FC-B, then proj-AB ----
            gpB = stack.enter_context(
                tc.tile_pool(name="gpB", bufs=1, side="right"))
            gTB2 = gpB.tile([128, 32, CH], bf16)
            x2Bp = stack.enter_context(
                tc.tile_pool(name="x2Bp", bufs=1, side="right"))
            x2s[1] = x2Bp.tile([128, KT, CH], bf16, name="x2B")
            prsb = stack.enter_context(tc.tile_pool(name="prsb", bufs=1))
            mlB_ctx = ExitStack()
            mlsbB = mlB_ctx.enter_context(tc.tile_pool(name="mlsbB", bufs=1))

            # zero gate that depends on attention-B: keeps the deferred
            # gelus (which only depend on gTA) out of the exp window
            nc.vector.tensor_scalar_mul(gate_t[:], attnTs[1][:, 0, 0:1], 0.0)
            for mt in range(32):     # deferred gelu for slot A (in-place)
                nc.scalar.activation(gTA[:, mt, :], gTA[:, mt, :], AF.Gelu,
                                     bias=gate_t[:])

            prps_ctx = ExitStack()
            prps = prps_ctx.enter_context(
                tc.tile_pool(name="prps", bufs=1, space="PSUM"))
            interleave(chain(ap_ln2_stream(1, mlsbB, gemmps),
                             fc_stream(1, gTB2, mlsbB, gemmps)),
                       proj_stream(0, gTA, prps, prsb), 24 / 48.0)
            mlB_ctx.close()

            # ---- window 4: proj-B ----
            for _ in proj_stream(1, gTB2, prps, prsb):
                pass
            prps_ctx.close()
            gemmps_ctx.close()

    nc.compile()
    return nc


def _host_prep(inputs):
    """Build the 8 per-core input maps."""
    x = np.asarray(inputs["x"], np.float32)
    ln1_g = np.asarray(inputs["ln1_g"], np.float32)
    ln1_b = np.asarray(inputs["ln1_b"], np.float32)
    ln2_g = np.asarray(inputs["ln2_g"], np.float32)
    ln2_b = np.asarray(inputs["ln2_b"], np.float32)

    # Fold LN1 gamma/beta into the QKV GEMM, and 1/sqrt(head_dim) into Q.
    w_attn_raw = np.asarray(inputs["w_attn"], np.float32)
    w_attn = (w_attn_raw * ln1_g[:, None]).copy()
    b_attn = (np.asarray(inputs["b_attn"], np.float32)
              + ln1_b @ w_attn_raw).copy()
    w_attn[:, :E] *= 0.125
    b_attn[:E] *= 0.125
    w_attn_bf = np.ascontiguousarray(w_attn.astype(BF))
    b_qk = np.ascontiguousarray(b_attn[:2 * E].reshape(2 * E, 1))
    b_v_bc = np.ascontiguousarray(np.broadcast_to(
        b_attn[2 * E:].reshape(1, 2, 8, D), (128, 2, 8, D)).astype(BF))

    # Fold LN2 gamma/beta into the FC GEMM.
    w_fc_raw = np.asarray(inputs["w_fc"], np.float32)
    w_fc = w_fc_raw * ln2_g[:, None]
    b_fc = np.asarray(inputs["b_fc"], np.float32) + ln2_b @ w_fc_raw

    # Swizzle every weight into its exact SBUF panel layout so each panel
    # DMA is one contiguous run per partition.
    F8 = ml_dtypes.float8_e4m3
    w_qk_sc = w_attn[:, :2 * E].copy()
    w_qk_sc[:, :E] *= QSC
    w_qk_sc[:, E:] *= KSC
    wqk_swz = np.ascontiguousarray(
        w_qk_sc.astype(F8).reshape(KT, 128, 8, 256).transpose(1, 2, 0, 3))
    wv_swz = np.ascontiguousarray(
        (w_attn[:, 2 * E:] * KSC).astype(F8)
        .reshape(KT, 128, 2, CH).transpose(1, 2, 0, 3))
    w_ap_bf = np.asarray(inputs["w_attnproj"], np.float32).astype(BF)
    wap_swz = np.ascontiguousarray(
        w_ap_bf.reshape(KT, 128, 4, 256).transpose(1, 2, 0, 3))
    wfc_swz = np.ascontiguousarray(
        w_fc.astype(BF).reshape(KT, 128, 8, CH).transpose(1, 2, 0, 3))
    wpr_swz = np.ascontiguousarray(
        np.asarray(inputs["w_proj"], np.float32).astype(BF)
        .reshape(8, 4, 128, E).transpose(2, 0, 1, 3))
    col = lambda v: np.ascontiguousarray(np.asarray(v, np.float32).reshape(-1, 1))
    b_ap = col(inputs["b_attnproj"])
    b_fc = col(b_fc)
    b_pr = col(inputs["b_proj"])

    # static diagonal pair masks (bf16 0/1, applied post-exp):
    # within a 512-chunk, kv tile t allows query col j iff j >= t*128 + p.
    j = np.arange(CH)[None, :]
    p = np.arange(128)[:, None]
    m01 = [np.where(j >= t * 128 + p, 1.0, 0.0).astype(np.float32)
           for t in range(4)]
    pm = np.stack([np.concatenate([m01[0], m01[1]], axis=1),
                   np.concatenate([m01[2], m01[3]], axis=1)])
    pm_bf = np.ascontiguousarray(pm.astype(BF))

    ON = (1.0, 0.0)
    OFF = (0.0, NEG)
    in_maps = []
    perms = []
    for core in range(8):
        b = core // 2
        z = core % 2
        blocks = [0, 3, 1, 2] if z == 0 else [1, 2, 0, 3]
        perms.append(blocks)
        cols = np.concatenate([np.arange(c * CH, (c + 1) * CH) for c in blocks])
        # x, transposed+permuted, bf16, swizzled [128, chunk, kt, 512]
        x_swz = np.ascontiguousarray(
            x[b].T[:, cols].astype(BF)
            .reshape(KT, 128, 4, CH).transpose(1, 2, 0, 3))
        # slot A: driven block = O1 (perm pos 2); allowed iff block(O1) < block(A)
        sa = ON if blocks[2] < blocks[0] else OFF
        # slot B: driven = A, O1, O2 (perm pos 0, 2, 3) vs chunk B
        sbs = [ON if blocks[i] < blocks[1] else OFF for i in (0, 2, 3)]
        f = np.float32
        in_maps.append({
            "x_swz": x_swz,
            "w_qk": wqk_swz, "w_v": wv_swz, "b_qk": b_qk,
            "b_v_bc": b_v_bc,
            "w_ap": wap_swz, "b_ap": b_ap,
            "w_fc": wfc_swz, "b_fc": b_fc, "w_proj": wpr_swz, "b_proj": b_pr,
            "pmask": pm_bf,
            "sA_scale": np.full((128, 1), sa[0], f),
            "sA_bias": np.full((128, 1), sa[1], f),
            "sB_scale": np.ascontiguousarray(
                np.tile(np.array([[s for s, _ in sbs]], f), (128, 1))),
            "sB_bias": np.ascontiguousarray(
                np.tile(np.array([[bb for _, bb in sbs]], f), (128, 1))),
        })
    return in_maps, perms


def _run(inputs, trace=False):
    from concourse.bass_utils import run_bass_kernel_spmd

    if "nc" not in _CACHE:
        _CACHE["nc"] = _build_program()
    nc = _CACHE["nc"]
    in_maps, perms = _host_prep(inputs)
    res = run_bass_kernel_spmd(nc, in_maps, list(range(8)), trace=trace)
    x = np.asarray(inputs["x"], np.float32)
    out = np.empty_like(x)
    for core in range(8):
        b = core // 2
        blocks = perms[core]
        oT = res.results[core]["outT"]
        cA, cB = blocks[0], blocks[1]
        out[b, cA * CH:(cA + 1) * CH, :] = oT[:, 0:CH].T
        out[b, cB * CH:(cB + 1) * CH, :] = oT[:, CH:2 * CH].T
    return out, res


def kernel(**inputs) -> np.ndarray:
    out, _ = _run(inputs, trace=False)
    return out

